# revision 8
# baseline (speedup 1.0000x reference)
"""Trainium2 Bass kernel for nn_ContinusConvolution (GNN message passing).

Math (see reference):
    P   = s_ij @ W_s                     # (B,N,NB,C)
    G'  = (m_ij * z_ij) @ W_z            # (B,N,NB,C)  [mask folded into z]
    S1  = sum_k P_k * G'_k               # (B,N,C)
    SG  = (sum_k m_k z_k) @ W_z          # (B,N,C)     [= sum_k G'_k]
    out = LayerNorm(S1 - s_i * SG) * gamma + beta

Device mapping (per core, nodes sharded 8 ways over B*N; 4 groups of 128
nodes per core):

  Feature-major compute. All activations are pre-transposed on the HOST
  (free) so every device DMA is linear:
    sT[g][p, f, k*128+n] = s_ij[node, k, f*128+p]     (bf16)
    zT[g][p, k*128+n]    = m*z_ij[node, k, p]         (bf16)
  Matmuls keep the WEIGHTS stationary (loaded once per chunk) and stream
  tokens as the moving operand:
    P^T[e] (128 feat x tokens) = sum_f Ws[f,e]^T @ sT[f]   (3 accumulating)
    G^T[e]                    = Wz[e]^T @ zT
  PSUM tiles are [128,1024] fp32 (2 banks), double buffered: 8 banks total.
  P*G product on DVE (PSUM x PSUM -> bf16 SBUF); the k-reduction (sum of 32
  [128,128] planes) is a binary tree on the Pool engine (SBUF bf16).
  SG^T comes from a tiny matmul of the host-reduced sum_k m z (transposed).
  preT = S1^T - s_i^T * SG^T goes back to node-major via a 128x128 DMA-xbar
  transpose (14ns/32x32-tile => 224ns each), then a standard bn_stats
  LayerNorm epilogue in node-major with linear output DMA.
"""

import contextlib

import numpy as np
import ml_dtypes

import concourse.bass as bass
import concourse.mybir as mybir
import concourse.tile as tile
from concourse import bacc
from concourse.bass_utils import run_bass_kernel_spmd

B, N, NB, C, CZ = 4, 1024, 32, 384, 128
EPS = 1e-6
NCORES = 8
NODES = B * N                      # 4096 total nodes
NPC = NODES // NCORES              # 512 nodes per core
PGROUP = 128                       # nodes per group (partition dim)
CE = C // 128                      # 3 output-feature chunks
TOK = PGROUP * NB                  # 4096 tokens per group (k-major: t=k*128+n)
KT = 4                             # token tiles per (e, group): 1024 cols each
TW = TOK // KT                     # 1024 tokens per tile (2 PSUM banks)

bf16 = ml_dtypes.bfloat16
dt = mybir.dt
ALU = mybir.AluOpType


def build_nc(groups=NPC // PGROUP, reps=1):
    nodes = groups * PGROUP
    nc = bacc.Bacc("TRN2", target_bir_lowering=False, debug=False)

    s_t = nc.declare_dram_parameter("s_t", [groups, 128, CE * TOK], dt.bfloat16, isOutput=False)
    z_t = nc.declare_dram_parameter("z_t", [groups, 128, TOK], dt.bfloat16, isOutput=False)
    szt = nc.declare_dram_parameter("szt", [128, nodes], dt.bfloat16, isOutput=False)
    sit = nc.declare_dram_parameter("sit", [groups, 128, CE * 128], dt.float32, isOutput=False)
    w_s = nc.declare_dram_parameter("w_s", [CE, 128, C], dt.bfloat16, isOutput=False)
    w_z = nc.declare_dram_parameter("w_z", [CZ, C], dt.bfloat16, isOutput=False)
    gmb = nc.declare_dram_parameter("gmb", [2, 128, C], dt.float32, isOutput=False)
    out = nc.declare_dram_parameter("out", [nodes, C], dt.float32, isOutput=True)

    with tile.TileContext(nc) as tc:
        with (
            tc.tile_pool(name="const", bufs=1) as cpool,
            tc.tile_pool(name="sT", bufs=2) as sT_pool,
            tc.tile_pool(name="zT", bufs=2) as zT_pool,
            tc.tile_pool(name="siT", bufs=2) as siT_pool,
            tc.tile_pool(name="prod", bufs=2) as prod_pool,
            tc.tile_pool(name="gsb", bufs=3) as gsb_pool,
            tc.tile_pool(name="tree", bufs=2) as tree_pool,
            tc.tile_pool(name="pre", bufs=2) as pre_pool,
            tc.tile_pool(name="epi", bufs=2) as epi_pool,
            tc.tile_pool(name="psum_p", bufs=2, space="PSUM") as p_pool,
            tc.tile_pool(name="psum_g", bufs=2, space="PSUM") as g_pool,
        ):
            # --- constants (once) ---
            wssb = cpool.tile([128, CE, C], dt.bfloat16)
            nc.sync.dma_start(out=wssb, in_=w_s[:].rearrange("e p d -> p e d"))
            wzsb = cpool.tile([128, C], dt.bfloat16)
            nc.sync.dma_start(out=wzsb, in_=w_z[:])
            sztsb = cpool.tile([128, nodes], dt.bfloat16)
            nc.sync.dma_start(out=sztsb, in_=szt[:])
            gam = cpool.tile([128, C], dt.float32)
            nc.sync.dma_start(out=gam, in_=gmb[0])
            bet = cpool.tile([128, C], dt.float32)
            nc.sync.dma_start(out=bet, in_=gmb[1])
            epst = cpool.tile([128, 1], dt.float32)
            nc.vector.memset(epst, EPS)

            def load_group(g):
                """Claim tiles + issue the big linear loads for group g (SP)."""
                sTg = sT_pool.tile([128, CE, TOK], dt.bfloat16, tag="sTg")
                nc.sync.dma_start(out=sTg, in_=s_t[g].rearrange("p (e t) -> p e t", e=CE))
                zTg = zT_pool.tile([128, TOK], dt.bfloat16, tag="zTg")
                nc.sync.dma_start(out=zTg, in_=z_t[g])
                siTg = siT_pool.tile([128, CE, 128], dt.float32, tag="siTg")
                nc.sync.dma_start(out=siTg, in_=sit[g].rearrange("p (e n) -> p e n", e=CE))
                return sTg, zTg, siTg

            loop_cm = tc.For_i(0, reps, 1) if reps > 1 else contextlib.nullcontext()
            with loop_cm:
              cur = load_group(0)
              for g in range(groups):
                gsl = slice(g * 128, (g + 1) * 128)
                # prefetch next group's loads ahead of this group's compute so
                # their SP-stream triggers fire immediately (buffer-free dep
                # only) and the transfers overlap with compute
                sTg, zTg, siTg = cur
                if g + 1 < groups:
                    cur = load_group(g + 1)

                pre = pre_pool.tile([128, C], dt.bfloat16, tag="pre")
                for e in range(CE):
                    esl = slice(e * 128, (e + 1) * 128)
                    prod = prod_pool.tile([128, TOK], dt.bfloat16)
                    for kt in range(KT):
                        Pp = p_pool.tile([128, TW], dt.float32)
                        Gp = g_pool.tile([128, TW], dt.float32)
                        for h in range(2):
                            hsl = slice(h * 512, (h + 1) * 512)
                            tsl = slice(kt * TW + h * 512, kt * TW + (h + 1) * 512)
                            for f in range(CE):
                                nc.tensor.matmul(
                                    Pp[:, hsl], wssb[:, f, esl], sTg[:, f, tsl],
                                    start=(f == 0), stop=(f == CE - 1),
                                )
                            nc.tensor.matmul(
                                Gp[:, hsl], wzsb[:, esl], zTg[:, tsl],
                                start=True, stop=True,
                            )
                        # PSUM->SBUF copy of G on ScalarE (DVE may read only
                        # one PSUM operand)
                        gsb = gsb_pool.tile([128, TW], dt.bfloat16)
                        nc.scalar.activation(
                            out=gsb, in_=Gp,
                            func=mybir.ActivationFunctionType.Copy, scale=1.0,
                        )
                        nc.vector.tensor_tensor(
                            out=prod[:, kt * TW:(kt + 1) * TW],
                            in0=Pp, in1=gsb, op=ALU.mult,
                        )
                    # k-reduction: sum 32 [128,128] planes, binary tree on Pool
                    r1 = tree_pool.tile([128, TOK // 2], dt.bfloat16, tag="r1")
                    nc.gpsimd.tensor_tensor(
                        out=r1, in0=prod[:, :TOK // 2], in1=prod[:, TOK // 2:],
                        op=ALU.add)
                    r2 = tree_pool.tile([128, TOK // 4], dt.bfloat16, tag="r2")
                    nc.gpsimd.tensor_tensor(
                        out=r2, in0=r1[:, :TOK // 4], in1=r1[:, TOK // 4:],
                        op=ALU.add)
                    r3 = tree_pool.tile([128, TOK // 8], dt.bfloat16, tag="r3")
                    nc.gpsimd.tensor_tensor(
                        out=r3, in0=r2[:, :TOK // 8], in1=r2[:, TOK // 8:],
                        op=ALU.add)
                    r4 = tree_pool.tile([128, TOK // 16], dt.bfloat16, tag="r4")
                    nc.gpsimd.tensor_tensor(
                        out=r4, in0=r3[:, :TOK // 16], in1=r3[:, TOK // 16:],
                        op=ALU.add)
                    s1t = tree_pool.tile([128, 128], dt.float32, tag="s1t")
                    nc.gpsimd.tensor_tensor(
                        out=s1t, in0=r4[:, :128], in1=r4[:, 128:], op=ALU.add)

                    # SG^T chunk + combine: preT = s1t - siT*SG
                    SGp = g_pool.tile([128, TW], dt.float32, tag="Gp")
                    nc.tensor.matmul(
                        SGp[:, :128], wzsb[:, esl], sztsb[:, gsl],
                        start=True, stop=True,
                    )
                    tmp = epi_pool.tile([128, 128], dt.float32, tag="tmp")
                    nc.vector.tensor_tensor(
                        out=tmp, in0=siTg[:, e, :], in1=SGp[:, :128], op=ALU.mult)
                    preT = pre_pool.tile([128, 128], dt.bfloat16, tag="preT")
                    nc.gpsimd.tensor_tensor(
                        out=preT, in0=s1t, in1=tmp, op=ALU.subtract)
                    # back to node-major [node, feat-chunk e]; issued on the
                    # Activation HWDGE stream so it never queues behind the
                    # multi-us group loads on SP
                    nc.scalar.dma_start_transpose(pre[:, esl], preT)

                # --- LayerNorm epilogue (node-major) ---
                stats = epi_pool.tile([128, 6], dt.float32, tag="stats")
                nc.vector.bn_stats(out=stats, in_=pre)
                mv = epi_pool.tile([128, 2], dt.float32, tag="mv")
                nc.vector.bn_aggr(out=mv, in_=stats)
                rstd = epi_pool.tile([128, 1], dt.float32, tag="rstd")
                nc.scalar.activation(
                    out=rstd, in_=mv[:, 1:2],
                    func=mybir.ActivationFunctionType.Sqrt,
                    bias=epst, scale=1.0,
                )
                nc.vector.reciprocal(out=rstd, in_=rstd)
                fin = epi_pool.tile([128, C], dt.float32, tag="fin")
                nc.vector.tensor_scalar(
                    out=fin, in0=pre,
                    scalar1=mv[:, 0:1], scalar2=rstd,
                    op0=ALU.subtract, op1=ALU.mult,
                )
                nc.gpsimd.tensor_tensor(out=fin, in0=fin, in1=gam, op=ALU.mult)
                nc.gpsimd.tensor_tensor(out=fin, in0=fin, in1=bet, op=ALU.add)
                nc.scalar.dma_start(out=out[gsl, :], in_=fin)

    nc.compile()
    return nc


def host_prep(s_i, s_ij, m_ij, z_ij, W_s, W_z, gamma, beta, groups=NPC // PGROUP):
    """Per-core input maps; all transposes happen here (host is free)."""
    nodes_pc = groups * PGROUP
    m_flat = m_ij.reshape(NODES, NB).astype(np.float32)
    z_m = z_ij.reshape(NODES, NB, CZ) * m_flat[:, :, None]      # mask folded
    sz = z_m.sum(axis=1)                                        # (NODES, CZ)

    w_s_h = np.ascontiguousarray(W_s.reshape(CE, 128, C)).astype(bf16)
    w_z_h = np.ascontiguousarray(W_z).astype(bf16)
    gmb_h = np.stack([
        np.broadcast_to(gamma.astype(np.float32), (128, C)),
        np.broadcast_to(beta.astype(np.float32), (128, C)),
    ]).copy()

    s_r = s_ij.reshape(NODES, NB, C)
    si_r = s_i.reshape(NODES, C)

    in_maps = []
    for c in range(NCORES):
        lo = c * NPC
        nsl = slice(lo, lo + nodes_pc)
        # sT[g, p, f, k, n] = s[g*128+n, k, f*128+p]
        s_c = s_r[nsl].reshape(groups, 128, NB, CE, 128)
        s_T = np.ascontiguousarray(s_c.transpose(0, 4, 3, 2, 1)).astype(bf16)
        # zT[g, p, k, n] = z_m[g*128+n, k, p]
        z_c = z_m[nsl].reshape(groups, 128, NB, CZ)
        z_T = np.ascontiguousarray(z_c.transpose(0, 3, 2, 1)).astype(bf16)
        # szT[p, node] (CZ on partitions)
        sz_T = np.ascontiguousarray(sz[nsl].T).astype(bf16)
        # siT[g, p, e, n] = s_i[g*128+n, e*128+p]
        si_c = si_r[nsl].reshape(groups, 128, CE, 128).astype(np.float32)
        si_T = np.ascontiguousarray(si_c.transpose(0, 3, 2, 1))
        in_maps.append({
            "s_t": s_T.reshape(groups, 128, CE * TOK),
            "z_t": z_T.reshape(groups, 128, TOK),
            "szt": sz_T,
            "sit": si_T.reshape(groups, 128, CE * 128),
            "w_s": w_s_h,
            "w_z": w_z_h,
            "gmb": gmb_h,
        })
    return in_maps


_NC_CACHE = {}


def _get_nc(groups, reps=1):
    key = (groups, reps)
    if key not in _NC_CACHE:
        _NC_CACHE[key] = build_nc(groups, reps=reps)
    return _NC_CACHE[key]


def kernel(s_i, s_ij, m_ij, z_ij, W_s, W_z, gamma, beta):
    s_i = np.asarray(s_i)
    s_ij = np.asarray(s_ij)
    m_ij = np.asarray(m_ij)
    z_ij = np.asarray(z_ij)
    W_s = np.asarray(W_s)
    W_z = np.asarray(W_z)
    gamma = np.asarray(gamma)
    beta = np.asarray(beta)

    nc = _get_nc(NPC // PGROUP)
    in_maps = host_prep(s_i, s_ij, m_ij, z_ij, W_s, W_z, gamma, beta)
    res = run_bass_kernel_spmd(
        nc, in_maps, list(range(NCORES)), trace=TRACE, **TRACE_KWARGS
    )
    global LAST_RESULTS
    LAST_RESULTS = res
    outs = [np.asarray(res.results[i]["out"]) for i in range(NCORES)]
    return np.concatenate(outs, axis=0).reshape(B, N, C).astype(np.float32)


TRACE = False
TRACE_KWARGS = {}
LAST_RESULTS = None
